# revision 22
# baseline (speedup 1.0000x reference)
"""Multi-head attention (B=2, N=2048, C=1024, H=16) on 8 Trainium2 NeuronCores.

Sharding: tensor-parallel over heads (2 heads/core) for qkv-proj + attention;
all-to-all of the attention output (4 half-batch collectives, pipelined under
attention), then each core runs the output projection over the full channel
dim for its token slices.  Host concatenates slices.

Per-core structure (heads A=2c, B=2c+1):
  x^T  [c_in, tok]       via HW DMA-transpose (bf16), hoisted up front
  q/k  [128, tok]        rows 0-63 head A dims, 64-127 head B dims (packed)
  S^T  [128, 1024]       per j-chunk: head A cols 0:512, head B 512:1024,
                         computed as TWO CONCURRENT 64x128 row-tiled matmuls
                         (K=64 per head -> PE row groups 0/1, disjoint banks)
  exp  one ACTIVATE per j-chunk over [128, 1024] PSUM (both heads);
                         S PSUM tiles ping-pong (bufs=2) so ScalarE stays
                         saturated - exp is the throughput floor (~147us)
  out_u^T [65, i] = [v|1].T @ expS  (row 64 = softmax denominator)
  normalize: reciprocal_approx_fast + 1x64 ones outer-product broadcast (PE)
             + DVE multiply reading the PSUM broadcast directly

The emission schedule interleaves qkv/oproj/vn/norm work between attention
j-chunk pairs so the PE never idles long enough to re-throttle (HAM) and the
ScalarE exp pipeline never starves.
"""

import numpy as np
import ml_dtypes
from collections import deque
from contextlib import ExitStack

import concourse.bass as bass
import concourse.tile as tile
from concourse import bacc, mybir
from concourse.bass_utils import run_bass_kernel_spmd
from concourse.masks import make_identity

BF16 = mybir.dt.bfloat16
F32 = mybir.dt.float32
EXP = mybir.ActivationFunctionType.Exp
NPBF16 = ml_dtypes.bfloat16

NCORES = 8
B, NSEQ, C, H, D = 2, 2048, 1024, 16, 64
T = B * NSEQ                 # 4096 flattened tokens
SCALE = D ** -0.5            # folded into the exp activation
NKC = C // 128               # 8 contraction chunks
ITILE = 512                  # query tile (free dim of S^T)
NI = NSEQ // ITILE           # 4 i-tiles per batch
NJ = NSEQ // 128             # 16 key chunks per batch
HALF = 1024                  # tokens per all-to-all (half batch)
TFRAG = HALF // NCORES       # 128 tokens per core per all-to-all
TSL = B * NSEQ // NCORES     # 512 output tokens per core

DEBUG_DUMP = False


def build_program():
    nc = bacc.Bacc("TRN2", target_bir_lowering=False, debug=False,
                   num_devices=NCORES)

    x_d = nc.dram_tensor("x", [T, C], BF16, kind="ExternalInput")
    wqk_d = nc.dram_tensor("wqk", [C, 256], BF16, kind="ExternalInput")
    wv_d = nc.dram_tensor("wv", [C, 128], BF16, kind="ExternalInput")
    wp_d = nc.dram_tensor("wproj", [C, C], BF16, kind="ExternalInput")
    bp_d = nc.dram_tensor("bproj", [1, C], BF16, kind="ExternalInput")
    y_d = nc.dram_tensor("y", [TSL, C], F32, kind="ExternalOutput")

    a2a_in = [nc.dram_tensor(f"a2a_in{q}", [NCORES * 128, TFRAG], BF16)
              for q in range(4)]
    a2a_out = [nc.dram_tensor(f"a2a_out{q}", [NCORES * 128, TFRAG], BF16)
               for q in range(4)]

    if DEBUG_DUMP:
        dbg_qk = nc.dram_tensor("dbg_qk", [128, 2 * NSEQ], BF16,
                                kind="ExternalOutput")
        dbg_vn = nc.dram_tensor("dbg_vn", [128, NJ * 130], BF16,
                                kind="ExternalOutput")
        dbg_ouc = nc.dram_tensor("dbg_ouc", [65, 8 * ITILE], F32,
                                 kind="ExternalOutput")
        dbg_outT = nc.dram_tensor("dbg_outT", [128, T], BF16,
                                  kind="ExternalOutput")
        dbg_a2a = nc.dram_tensor("dbg_a2a", [2048, TFRAG], BF16,
                                 kind="ExternalOutput")

    with tile.TileContext(nc) as tc, ExitStack() as ctx:
        ep = ctx.enter_context

        consts = ep(tc.tile_pool(name="consts", bufs=1))
        p_exp = ep(tc.tile_pool(name="exps", bufs=3))
        p_ouc = ep(tc.tile_pool(name="ouc", bufs=6))
        p_small = ep(tc.tile_pool(name="small", bufs=4))
        p_ots = ep(tc.tile_pool(name="ots", bufs=2))
        p_y = ep(tc.tile_pool(name="ysb", bufs=2))
        ps_s = ep(tc.tile_pool(name="pss", bufs=2, space="PSUM"))
        ps_ou = ep(tc.tile_pool(name="psou", bufs=2, space="PSUM"))
        ps_mm = ep(tc.tile_pool(name="psmm", bufs=2, space="PSUM"))

        # ---- weights / constants to SBUF (batched 3D-AP DMAs) ----
        wqk_sb = consts.tile([128, NKC * 256], BF16, name="wqk_sb")
        wv_sb = consts.tile([128, NKC * 128], BF16, name="wv_sb")
        wp_sb = consts.tile([128, NKC * C], BF16, name="wp_sb")
        bp_sb = consts.tile([1, C], BF16, name="bp_sb")

        # x^T: one tile per batch, layout [:, c*2048 + t]; transposes in
        # [1024, 128] blocks so the first qkv matmuls start early.
        xt = [consts.tile([128, NKC * NSEQ], BF16, name=f"xt{b}")
              for b in range(B)]

        def load_weights_qkv():
            nc.sync.dma_start(
                out=wqk_sb[:].rearrange("p (c n) -> p c n", c=NKC),
                in_=wqk_d[:, :].rearrange("(c p) n -> p c n", p=128))
            nc.sync.dma_start(
                out=wv_sb[:].rearrange("p (c n) -> p c n", c=NKC),
                in_=wv_d[:, :].rearrange("(c p) n -> p c n", p=128))

        def load_weights_proj(eng):
            eng.dma_start(
                out=wp_sb[:].rearrange("p (c n) -> p c n", c=NKC),
                in_=wp_d[:, :].rearrange("(c p) n -> p c n", p=128))
            eng.dma_start(out=bp_sb[:], in_=bp_d[0:1, :])

        def load_xt(b, tp, eng):
            for c in range(NKC):
                eng.dma_start_transpose(
                    xt[b][:, c * NSEQ + tp * 1024: c * NSEQ + (tp + 1) * 1024],
                    x_d[b * NSEQ + tp * 1024: b * NSEQ + (tp + 1) * 1024,
                        c * 128:(c + 1) * 128])

        # startup order: what the first matmuls need, first
        load_xt(0, 0, nc.sync)
        load_weights_qkv()
        load_xt(0, 1, nc.sync)
        load_xt(1, 0, nc.sync)
        load_xt(1, 1, nc.sync)
        load_weights_proj(nc.sync)

        ident = consts.tile([128, 128], BF16, name="ident")
        make_identity(nc, ident[:])
        ones1 = consts.tile([65, 64], BF16, name="ones1")
        nc.vector.memset(ones1[:], 1.0)
        onesc = consts.tile([1, 128], BF16, name="onesc")
        nc.vector.memset(onesc[:], 1.0)

        # bias broadcast [128, C] f32, computed once via 1x128 outer product
        bias_bc = consts.tile([128, C], F32, name="bias_bc")

        def make_bias_bc():
            for n in range(2):
                bps = ps_mm.tile([128, ITILE], F32, tag="mm", name="bps")
                nc.tensor.matmul(bps[:], onesc[:],
                                 bp_sb[:, n * ITILE:(n + 1) * ITILE],
                                 start=True, stop=True)
                nc.vector.tensor_copy(bias_bc[:, n * ITILE:(n + 1) * ITILE],
                                      bps[:])

        # ---- persistent per-batch / per-chunk state ----
        # per-head q/k with the dead 64 rows zeroed once (K=128 matmuls keep
        # the PE in plain 128x128 mode -- mode switches corrupt in-flight MMs)
        qz = [[consts.tile([128, NSEQ], BF16, name=f"qz{b}{h}")
               for h in range(2)] for b in range(B)]
        kz = [[consts.tile([128, NSEQ], BF16, name=f"kz{b}{h}")
               for h in range(2)] for b in range(B)]
        for b in range(B):
            nc.vector.memset(qz[b][0][64:128, :], 0.0)
            nc.vector.memset(kz[b][0][64:128, :], 0.0)
            nc.vector.memset(qz[b][1][0:64, :], 0.0)
            nc.vector.memset(kz[b][1][0:64, :], 0.0)
        vT = [consts.tile([128, NSEQ], BF16, name=f"vT{b}") for b in range(B)]
        # vn[j]: [v_A(64) | 1 | v_B(64) | 1 | zeros(63)]; constants written once
        vn = [consts.tile([128, 193], BF16, name=f"vn{j}") for j in range(NJ)]
        for j in range(NJ):
            nc.vector.memset(vn[j][:, 64:65], 1.0)
            nc.vector.memset(vn[j][:, 129:130], 1.0)
            nc.vector.memset(vn[j][:, 130:193], 0.0)
        # normalized attention output, per head (partitions 0-63)
        outT = [consts.tile([64, T], BF16, name=f"outT{h}") for h in range(2)]

        def xts(b, u, c):
            return xt[b][:, c * NSEQ + u * ITILE: c * NSEQ + (u + 1) * ITILE]

        # ---- qkv projection: one (w, u) unit = 8 matmuls + 1 evac ----
        def qkv_unit(b, tp, w, uu):
            def emit():
                u = 2 * tp + uu
                usl = slice(u * ITILE, (u + 1) * ITILE)
                pst = ps_mm.tile([128, ITILE], F32, tag="mm", name="pst")
                for c in range(NKC):
                    if w < 2:
                        lhsT = wqk_sb[:, c * 256 + w * 128:
                                      c * 256 + (w + 1) * 128]
                    else:
                        lhsT = wv_sb[:, c * 128:(c + 1) * 128]
                    nc.tensor.matmul(pst[:], lhsT, xts(b, u, c),
                                     start=(c == 0), stop=(c == NKC - 1))
                if w == 2:
                    nc.vector.tensor_copy(vT[b][:, usl], pst[:])
                else:
                    dst = (qz, kz)[w][b]
                    nc.vector.tensor_copy(dst[0][0:64, usl], pst[0:64, :])
                    nc.vector.tensor_copy(dst[1][64:128, usl],
                                          pst[64:128, :])
            return emit

        def qkv_units(b, tp, ws=(0, 1, 2)):
            return [qkv_unit(b, tp, w, uu) for w in ws for uu in range(2)]

        # ---- vn construction: one unit = 2 transposes + 4 copies ----
        def vn_unit(b, tcn0):
            def emit():
                for tcn in (tcn0, tcn0 + 1):
                    vtr = ps_mm.tile([128, 128], BF16, tag="mm", name="vtr")
                    nc.tensor.transpose(vtr[:],
                                        vT[b][:, tcn * 128:(tcn + 1) * 128],
                                        ident[:])
                    nc.vector.tensor_copy(vn[tcn][:, 0:64], vtr[:, 0:64])
                    nc.vector.tensor_copy(vn[tcn][:, 65:129], vtr[:, 64:128])
            return emit

        def vn_units(b, tcns):
            return [vn_unit(b, t0) for t0 in tcns]

        # ---- attention ----
        outUc = {}

        def attn_pairs(b, i):
            """Generator: one yield per j-chunk pair (8 per i-tile)."""
            isl = slice(i * ITILE, (i + 1) * ITILE)
            outu = [ps_ou.tile([128, ITILE], F32, tag="ou", name="outu")
                    for _ in range(2)]
            for g in range(NJ // 2):
                sts = []
                for jj in (2 * g, 2 * g + 1):
                    s_t = ps_s.tile([128, 1024], F32, tag="s", name="s_t")
                    for h in range(2):
                        nc.tensor.matmul(
                            s_t[:, h * ITILE:(h + 1) * ITILE],
                            kz[b][h][:, jj * 128:(jj + 1) * 128],
                            qz[b][h][:, isl],
                            start=True, stop=True)
                    sts.append(s_t)
                exs = []
                for k in range(2):
                    ex = p_exp.tile([128, 1024], BF16, tag="ex", name="ex")
                    nc.scalar.activation(ex[:], sts[k][:], EXP, scale=SCALE)
                    exs.append(ex)
                for k, jj in enumerate((2 * g, 2 * g + 1)):
                    for h in range(2):
                        nc.tensor.matmul(
                            outu[h][:],
                            vn[jj][:, h * 65: h * 65 + 128],
                            exs[k][:, h * ITILE:(h + 1) * ITILE],
                            start=(jj == 0), stop=(jj == NJ - 1))
                yield
            for h in range(2):
                ouc = p_ouc.tile([65, ITILE], F32, tag="ouc", name="ouc")
                nc.vector.tensor_copy(ouc[:], outu[h][0:65, :])
                outUc[(b, i, h)] = ouc
                if DEBUG_DUMP and b == 0:
                    sl = (i * 2 + h) * ITILE
                    nc.sync.dma_start(out=dbg_ouc[:, sl:sl + ITILE],
                                      in_=ouc[:])

        normSt = {}

        def norm_rcp(b, i, h):
            """DVE-only part: reciprocal of the denominator + bf16 cast.
            No PE instructions, so it never blocks the tensor queue."""
            def emit():
                ouc = outUc.pop((b, i, h))
                rcp = p_small.tile([65, ITILE], F32, tag="rcp", name="rcp")
                nc.vector.reciprocal(rcp[64:65, :], ouc[64:65, :])
                rcpb = p_small.tile([65, ITILE], BF16, tag="rcpb", name="rcpb")
                nc.vector.tensor_copy(rcpb[64:65, :], rcp[64:65, :])
                normSt[(b, i, h)] = (ouc, rcpb)
            return emit

        def norm_mul(b, i, h):
            """PE broadcast of 1/den + DVE multiply; schedule a few slots
            after norm_rcp so the PE never waits on the reciprocal chain."""
            def emit():
                t0 = b * NSEQ
                ouc, rcpb = normSt.pop((b, i, h))
                bc_ps = ps_mm.tile([128, ITILE], F32, tag="mm", name="bcps")
                nc.tensor.matmul(bc_ps[0:64, :], ones1[64:65, :],
                                 rcpb[64:65, :], start=True, stop=True)
                nc.vector.tensor_mul(
                    outT[h][0:64, t0 + i * ITILE: t0 + (i + 1) * ITILE],
                    ouc[0:64, :], bc_ps[0:64, :])
            return emit

        def norm(b, i, h):
            def emit():
                norm_rcp(b, i, h)()
                norm_mul(b, i, h)()
            return emit

        def stage_a2a(q):
            b, half = q // 2, q % 2
            hs = b * NSEQ + half * HALF
            for h in range(2):
                nc.sync.dma_start(
                    out=a2a_in[q][:, :].rearrange("(s x) t -> x s t",
                                                  s=NCORES)[h * 64:
                                                            (h + 1) * 64],
                    in_=outT[h][:, hs:hs + HALF].rearrange("p (s t) -> p s t",
                                                           s=NCORES))
            nc.gpsimd.collective_compute(
                "AllToAll", mybir.AluOpType.bypass,
                replica_groups=[list(range(NCORES))],
                ins=[a2a_in[q][:, :]], outs=[a2a_out[q][:, :]])

        def oproj(q):
            def emit():
                b, half = q // 2, q % 2
                ots = p_ots.tile([128, NCORES * TFRAG], BF16, tag="ots",
                                 name="ots")
                nc.sync.dma_start(
                    out=ots[:].rearrange("p (s t) -> p s t", s=NCORES),
                    in_=a2a_out[q][:, :].rearrange("(s p) t -> p s t",
                                                   s=NCORES))
                y_ps = [ps_mm.tile([128, ITILE], F32, tag="mm", name="yps")
                        for _ in range(2)]
                for s in range(NKC):
                    for n in range(2):
                        nc.tensor.matmul(
                            y_ps[n][:],
                            ots[:, s * TFRAG:(s + 1) * TFRAG],
                            wp_sb[:, s * C + n * ITILE: s * C + (n + 1) * ITILE],
                            start=(s == 0), stop=(s == NKC - 1))
                y_sb = p_y.tile([128, C], F32, tag="y", name="ysb")
                for n in range(2):
                    nc.vector.tensor_add(y_sb[:, n * ITILE:(n + 1) * ITILE],
                                         y_ps[n][:],
                                         bias_bc[:, n * ITILE:(n + 1) * ITILE])
                yr0 = b * (TSL // B) + half * TFRAG
                nc.sync.dma_start(out=y_d[yr0: yr0 + TFRAG, :], in_=y_sb[:])
            return emit

        def drive(gen, fillers):
            """Interleave: one filler unit emitted BEFORE each attention
            j-pair; leftovers drain after the i-tile. None = empty slot."""
            fl = deque(fillers)
            while True:
                if fl:
                    f = fl.popleft()
                    if f is not None:
                        f()
                try:
                    next(gen)
                except StopIteration:
                    break
            while fl:
                f = fl.popleft()
                if f is not None:
                    f()

        # ---- emission schedule ----
        make_bias_bc()
        for f in qkv_units(0, 0):
            f()
        for f in vn_units(0, range(0, 8, 2)):
            f()

        drive(attn_pairs(0, 0),
              qkv_units(0, 1, ws=(2, 1)) + vn_units(0, range(8, 16, 2))
              + qkv_units(0, 1, ws=(0,)))
        if DEBUG_DUMP:
            nc.sync.dma_start(out=dbg_qk[0:64, 0:NSEQ], in_=qz[0][0][0:64, :])
            nc.sync.dma_start(out=dbg_qk[64:128, 0:NSEQ],
                              in_=qz[0][1][64:128, :])
            nc.sync.dma_start(out=dbg_qk[0:64, NSEQ:], in_=kz[0][0][0:64, :])
            nc.sync.dma_start(out=dbg_qk[64:128, NSEQ:],
                              in_=kz[0][1][64:128, :])
            for j in range(NJ):
                nc.sync.dma_start(out=dbg_vn[:, j * 130:j * 130 + 130],
                                  in_=vn[j][:, 0:130])
        def seq(*fns):
            def emit():
                for f in fns:
                    f()
            return emit

        def a2a_unit(q):
            def emit():
                stage_a2a(q)
            return emit

        drive(attn_pairs(0, 1),
              [norm_rcp(0, 0, 0), norm_rcp(0, 0, 1), None,
               norm_mul(0, 0, 0), norm_mul(0, 0, 1)])
        norm_rcp(0, 1, 0)(); norm_rcp(0, 1, 1)()
        norm_mul(0, 1, 0)(); norm_mul(0, 1, 1)()
        stage_a2a(0)
        drive(attn_pairs(0, 2), qkv_units(1, 0))
        drive(attn_pairs(0, 3),
              [norm_rcp(0, 2, 0), norm_rcp(0, 2, 1)]
              + vn_units(1, range(0, 8, 2))
              + [norm_mul(0, 2, 0), norm_mul(0, 2, 1)])
        norm_rcp(0, 3, 0)(); norm_rcp(0, 3, 1)()
        norm_mul(0, 3, 0)(); norm_mul(0, 3, 1)()
        stage_a2a(1)
        drive(attn_pairs(1, 0),
              qkv_units(1, 1, ws=(2, 1)) + vn_units(1, range(8, 16, 2))
              + qkv_units(1, 1, ws=(0,)) + [oproj(0)])
        drive(attn_pairs(1, 1),
              [norm_rcp(1, 0, 0), norm_rcp(1, 0, 1), None,
               norm_mul(1, 0, 0), norm_mul(1, 0, 1), None, oproj(1)])
        norm_rcp(1, 1, 0)(); norm_rcp(1, 1, 1)()
        norm_mul(1, 1, 0)(); norm_mul(1, 1, 1)()
        stage_a2a(2)
        drive(attn_pairs(1, 2), [])
        drive(attn_pairs(1, 3),
              [norm_rcp(1, 2, 0), norm_rcp(1, 2, 1), None,
               norm_mul(1, 2, 0), norm_mul(1, 2, 1), None, oproj(2)])
        norm_rcp(1, 3, 0)(); norm_rcp(1, 3, 1)()
        norm_mul(1, 3, 0)(); norm_mul(1, 3, 1)()
        if DEBUG_DUMP:
            nc.sync.dma_start(out=dbg_outT[0:64, :], in_=outT[0][:])
            nc.sync.dma_start(out=dbg_outT[64:128, :], in_=outT[1][:])
            nc.sync.dma_start(out=dbg_a2a[0:1024, :], in_=a2a_in[2][:, :])
            nc.sync.dma_start(out=dbg_a2a[1024:2048, :], in_=a2a_out[2][:, :])
        stage_a2a(3)
        oproj(3)()

    nc.compile()
    return nc


_NC = None


def _get_nc():
    global _NC
    if _NC is None:
        _NC = build_program()
    return _NC


def prep_in_maps(x, w_qkv, w_proj, b_proj):
    x_bf = np.ascontiguousarray(np.asarray(x, dtype=np.float32).reshape(T, C)
                                ).astype(NPBF16)
    w_qkv = np.asarray(w_qkv, dtype=np.float32)
    w_proj = np.asarray(w_proj, dtype=np.float32)
    b_proj = np.asarray(b_proj, dtype=np.float32)
    wp_bf = np.ascontiguousarray(w_proj).astype(NPBF16)
    bp_bf = b_proj.reshape(1, C).astype(NPBF16)

    q_w, k_w, v_w = w_qkv[:, 0:C], w_qkv[:, C:2 * C], w_qkv[:, 2 * C:3 * C]
    in_maps = []
    for c in range(NCORES):
        hA, hB = 2 * c, 2 * c + 1
        sA, sB = slice(hA * D, (hA + 1) * D), slice(hB * D, (hB + 1) * D)
        wqk_c = np.concatenate([q_w[:, sA], q_w[:, sB], k_w[:, sA], k_w[:, sB]],
                               axis=1).astype(NPBF16)
        wv_c = np.concatenate([v_w[:, sA], v_w[:, sB]], axis=1).astype(NPBF16)
        in_maps.append({"x": x_bf, "wqk": np.ascontiguousarray(wqk_c),
                        "wv": np.ascontiguousarray(wv_c), "wproj": wp_bf,
                        "bproj": bp_bf})
    return in_maps


def assemble(results):
    y = np.empty((T, C), dtype=np.float32)
    for c in range(NCORES):
        yc = results[c]["y"]
        for b in range(B):
            for half in range(2):
                g0 = b * NSEQ + half * HALF + c * TFRAG
                r0 = b * (TSL // B) + half * TFRAG
                y[g0: g0 + TFRAG, :] = yc[r0: r0 + TFRAG, :]
    return y.reshape(B, NSEQ, C)


def run(in_maps, trace=False):
    nc = _get_nc()
    return run_bass_kernel_spmd(nc, in_maps, core_ids=list(range(NCORES)),
                                trace=trace)


def kernel(x, w_qkv, w_proj, b_proj):
    res = run(prep_in_maps(x, w_qkv, w_proj, b_proj))
    return assemble(res.results)
